# revision 12
# baseline (speedup 1.0000x reference)
"""Trainium2 Bass kernel for nn_FastAttention: out = v + q @ (k^T @ v) per (b,h).

Full shapes: q,k,v [B=2, H=16, S=4096, D=128] f32.
Sharding: B*H = 32 pairs split across 8 cores -> 4 pairs/core, no collectives.

All HBM IO is bf16 (inputs downcast on host, output upcast on host): this
kernel is a pure stream (every byte of q,k,v read once, out written once), so
bytes are the roofline. bf16 halves traffic to 16MB/core (~44.7us at
358GB/s/core); f32 PSUM accumulation keeps max-rel error ~4.5e-3, inside the
2e-2 gate (verified bit-exact vs a host simulation; fp8 q fails at 2.9e-2).

Per (b,h) pair on-core:
  phase A: kv[d,e] = sum_s k[s,d] v[s,e]    (32 accumulating 128-row matmuls)
  phase T: qT[d,s] = q[s,d]^T               (PE transpose via bf16 identity)
  phase B: out[s,e] = v[s,e] + sum_d qT[d,s] kv[d,e]

Schedule notes (from perfetto traces):
  - SBUF layout tile[p, n*128+d] = x[32p+n, d]: each load/store is one
    whole-tile DMA, 8KB contiguous per partition (max descriptor size, near
    line rate); a matmul "chunk" is a plain column slice of the tile.
  - Loads AND stores all trigger from the Sync sequencer, stores emitted
    after every load: DIRECT2D triggers execute in order, so every store
    descriptor lands in the DMA queue FIFOs behind every load descriptor.
    Loads therefore finish ~3us earlier and the post-last-load compute tail
    (T/B/add chain of the last pair) overlaps the store drain instead of
    extending it. o_sb has 4 bufs so early pairs' outputs wait in SBUF.
  - T(g+1) is emitted before B(g) so the PSUM->SBUF qT copy (ACT) hides
    behind the next group's transposes; qT copies on ACT, v-adds on DVE so
    neither in-order queue blocks the other.
  - Transpose output dtype must match its input (bf16), so qT PSUM tiles are
    bf16 (half a bank); kv/out accumulate in f32 PSUM and downcast on copy.
"""

import sys

if "/opt/trn_rl_repo" not in sys.path:
    sys.path.insert(0, "/opt/trn_rl_repo")

import ml_dtypes
import numpy as np

import concourse.bass as bass
import concourse.mybir as mybir
import concourse.tile as tile
from concourse import bacc
from concourse.bass import ts
from concourse.bass_utils import run_bass_kernel_spmd
from concourse.masks import make_identity

B, H, S, D = 2, 16, 4096, 128
N_CORES = 8
PAIRS = (B * H) // N_CORES  # 4
F32 = mybir.dt.float32
BF16 = mybir.dt.bfloat16


def build_nc(pairs=PAIRS, s=S):
    nc = bacc.Bacc(
        "TRN2", target_bir_lowering=False, debug=False, num_devices=N_CORES
    )
    q = nc.dram_tensor("q", [pairs, s, D], BF16, kind="ExternalInput").ap()
    k = nc.dram_tensor("k", [pairs, s, D], BF16, kind="ExternalInput").ap()
    v = nc.dram_tensor("v", [pairs, s, D], BF16, kind="ExternalInput").ap()
    out = nc.dram_tensor("out", [pairs, s, D], BF16, kind="ExternalOutput").ap()

    nch = s // 128  # s-chunks per pair
    gsz = 4  # chunks per psum group (512 free-dim)
    ngrp = nch // gsz

    with tile.TileContext(nc) as tc:
        with (
            tc.tile_pool(name="const", bufs=1) as cpool,
            tc.tile_pool(name="io", bufs=2) as io,
            tc.tile_pool(name="os", bufs=4) as os_pool,
            tc.tile_pool(name="pskv", bufs=2, space="PSUM") as pskv,
            tc.tile_pool(name="psq", bufs=3, space="PSUM") as psq,
            tc.tile_pool(name="pso", bufs=3, space="PSUM") as pso,
        ):
            ident = cpool.tile([128, 128], BF16)
            make_identity(nc, ident[:])

            stores = []  # deferred (dram AP, o_sb tile) per pair
            for p in range(pairs):
                k_sb = io.tile([128, s], BF16, tag="k")
                v_sb = io.tile([128, s], BF16, tag="v")
                q_sb = io.tile([128, s], BF16, tag="q")
                qT_sb = io.tile([128, s], BF16, tag="qT")
                kv_sb = io.tile([128, 128], BF16, tag="kv")
                o_sb = os_pool.tile([128, s], BF16, tag="o")

                k3 = k[p].rearrange("(p n) d -> p n d", p=128)
                v3 = v[p].rearrange("(p n) d -> p n d", p=128)
                q3 = q[p].rearrange("(p n) d -> p n d", p=128)
                k_t3 = k_sb[:].rearrange("p (n d) -> p n d", d=128)
                v_t3 = v_sb[:].rearrange("p (n d) -> p n d", d=128)
                q_t3 = q_sb[:].rearrange("p (n d) -> p n d", d=128)
                nc.sync.dma_start(out=k_t3[:, ts(0, nch)], in_=k3[:, ts(0, nch)])
                nc.sync.dma_start(out=v_t3[:, ts(0, nch)], in_=v3[:, ts(0, nch)])
                nc.sync.dma_start(out=q_t3[:, ts(0, nch)], in_=q3[:, ts(0, nch)])

                # phase A: kv[d,e] accumulated over s-chunks
                kv_ps = pskv.tile([128, 128], F32, tag="kv_ps")
                for n in range(nch):
                    nc.tensor.matmul(
                        kv_ps[:],
                        lhsT=k_sb[:, ts(n, 128)],
                        rhs=v_sb[:, ts(n, 128)],
                        start=(n == 0),
                        stop=(n == nch - 1),
                    )
                nc.vector.tensor_copy(kv_sb[:], kv_ps[:])

                def emit_T(g):
                    # transpose output dtype must match input (bf16 PSUM)
                    qt_ps = psq.tile([128, gsz * 128], BF16, tag="qt_ps")
                    for j in range(gsz):
                        n = g * gsz + j
                        nc.tensor.transpose(
                            qt_ps[:, ts(j, 128)], q_sb[:, ts(n, 128)], ident[:]
                        )
                    nc.scalar.copy(qT_sb[:, ts(g, gsz * 128)], qt_ps[:])

                def emit_B(g):
                    o_ps = pso.tile([128, gsz * 128], F32, tag="o_ps")
                    for j in range(gsz):
                        n = g * gsz + j
                        nc.tensor.matmul(
                            o_ps[:, ts(j, 128)],
                            lhsT=qT_sb[:, ts(n, 128)],
                            rhs=kv_sb[:],
                            start=True,
                            stop=True,
                        )
                    nc.vector.tensor_add(
                        o_sb[:, ts(g, gsz * 128)],
                        o_ps[:],
                        v_sb[:, ts(g, gsz * 128)],
                    )

                emit_T(0)
                for g in range(ngrp):
                    if g + 1 < ngrp:
                        emit_T(g + 1)
                    emit_B(g)

                stores.append((out[p].rearrange("(p n) d -> p n d", p=128), o_sb))

            # stores, emitted after ALL load triggers on the same (in-order)
            # Sync sequencer: their descriptors queue behind every load.
            # Last pair in halves so the final add chain overlaps the drain.
            for p, (o3, o_sb) in enumerate(stores):
                o_t3 = o_sb[:].rearrange("p (n d) -> p n d", d=128)
                if p < pairs - 1:
                    hs = bass.ds(0, nch)
                    nc.sync.dma_start(out=o3[:, hs], in_=o_t3[:, hs])
                else:
                    for h in range(2):
                        hs = bass.ds(h * (nch // 2), nch // 2)
                        nc.sync.dma_start(out=o3[:, hs], in_=o_t3[:, hs])
    nc.finalize()
    return nc


def kernel(q, k, v, _trace=False):
    bf16 = ml_dtypes.bfloat16
    q = np.ascontiguousarray(np.asarray(q, dtype=np.float32).astype(bf16)).reshape(
        B * H, S, D
    )
    k = np.ascontiguousarray(np.asarray(k, dtype=np.float32).astype(bf16)).reshape(
        B * H, S, D
    )
    v = np.ascontiguousarray(np.asarray(v, dtype=np.float32).astype(bf16)).reshape(
        B * H, S, D
    )

    nc = build_nc()
    in_maps = [
        {
            "q": q[i * PAIRS : (i + 1) * PAIRS],
            "k": k[i * PAIRS : (i + 1) * PAIRS],
            "v": v[i * PAIRS : (i + 1) * PAIRS],
        }
        for i in range(N_CORES)
    ]
    res = run_bass_kernel_spmd(nc, in_maps, core_ids=list(range(N_CORES)))
    full = np.concatenate([res.results[i]["out"] for i in range(N_CORES)], axis=0)
    out = full.reshape(B, H, S, D).astype(np.float32)
    if _trace:
        tres = [
            run_bass_kernel_spmd(
                nc,
                in_maps,
                core_ids=list(range(N_CORES)),
                trace=True,
                trace_cores=list(range(N_CORES)),
            )
            for _ in range(3)
        ]
        return out, tres
    return out


# revision 18
# speedup vs baseline: 1.0839x; 1.0839x over previous
"""Trainium2 Bass kernel for nn_FastAttention: out = v + q @ (k^T @ v) per (b,h).

Full shapes: q,k,v [B=2, H=16, S=4096, D=128] f32.
Sharding: B*H = 32 pairs split across 8 cores -> 4 pairs/core, no collectives.

All HBM IO is bf16 (inputs downcast on host, output upcast on host): this
kernel is a pure stream (every byte of q,k,v read once, out written once), so
bytes are the roofline. bf16 halves traffic to 16MB/core (~44.7us at
358GB/s/core); f32 PSUM accumulation keeps max-rel error ~4.5e-3, inside the
2e-2 gate (verified bit-exact vs a host simulation; fp8 q fails at 2.9e-2).

Per (b,h) pair on-core:
  phase A: kv[d,e] = sum_s k[s,d] v[s,e]    (32 accumulating 128-row matmuls)
  phase T: qT[d,s] = q[s,d]^T               (PE transpose via bf16 identity)
  phase B: out[s,e] = v[s,e] + sum_d qT[d,s] kv[d,e]

Schedule notes (from perfetto traces; fixed NEFF envelope is ~13.7us and the
16MB stream runs ~40.5us at ~395GB/s with 8KB descriptors):
  - SBUF layout tile[p, n*128+d] = x[32p+n, d]: each load/store is one
    whole-tile DMA, 8KB contiguous per partition (max descriptor size, near
    line rate); a matmul "chunk" is a plain column slice of the tile.
  - Loads AND stores all trigger from the Sync sequencer, stores emitted
    after every load: DIRECT2D triggers execute in order, so every store
    descriptor lands in the DMA queue FIFOs behind every load descriptor.
    Loads therefore finish earlier and the post-last-load compute tail of
    the last pair overlaps the store drain instead of extending it. o_sb
    has 4 bufs so early pairs' outputs wait in SBUF; k/v/q/qT have 3 so
    load triggers never stall on a WAR against 2-pairs-ago compute.
  - T(g+1) is emitted before B(g) so the PSUM->SBUF qT copy (ACT) hides
    behind the next group's transposes; qT copies on ACT, v-adds on DVE so
    neither in-order queue blocks the other (gpsimd cannot access PSUM, so
    only these two engines can drain PSUM). The LAST pair's q arrives in
    halves so its copy/add chains start at the half-way mark, compressing
    the chain that gates the final stores.
  - Transpose output dtype must match its input (bf16), so qT PSUM tiles are
    bf16 (half a bank); kv/out accumulate in f32 PSUM and downcast on copy.
"""

import sys

if "/opt/trn_rl_repo" not in sys.path:
    sys.path.insert(0, "/opt/trn_rl_repo")

import ml_dtypes
import numpy as np

import concourse.bass as bass
import concourse.mybir as mybir
import concourse.tile as tile
from concourse import bacc
from concourse.bass import ts
from concourse.bass_utils import run_bass_kernel_spmd
from concourse.masks import make_identity

B, H, S, D = 2, 16, 4096, 128
N_CORES = 8
PAIRS = (B * H) // N_CORES  # 4
F32 = mybir.dt.float32
BF16 = mybir.dt.bfloat16


def build_nc(pairs=PAIRS, s=S):
    nc = bacc.Bacc(
        "TRN2", target_bir_lowering=False, debug=False, num_devices=N_CORES
    )
    q = nc.dram_tensor("q", [pairs, s, D], BF16, kind="ExternalInput").ap()
    k = nc.dram_tensor("k", [pairs, s, D], BF16, kind="ExternalInput").ap()
    v = nc.dram_tensor("v", [pairs, s, D], BF16, kind="ExternalInput").ap()
    out = nc.dram_tensor("out", [pairs, s, D], BF16, kind="ExternalOutput").ap()

    nch = s // 128  # s-chunks per pair
    gsz = 4  # chunks per psum group (512 free-dim)
    ngrp = nch // gsz

    with tile.TileContext(nc) as tc:
        with (
            tc.tile_pool(name="const", bufs=1) as cpool,
            tc.tile_pool(name="io", bufs=3) as io,
            tc.tile_pool(name="os", bufs=4) as os_pool,
            tc.tile_pool(name="pskv", bufs=2, space="PSUM") as pskv,
            tc.tile_pool(name="psq", bufs=3, space="PSUM") as psq,
            tc.tile_pool(name="pso", bufs=3, space="PSUM") as pso,
        ):
            ident = cpool.tile([128, 128], BF16)
            make_identity(nc, ident[:])

            stores = []  # deferred (dram AP, o_sb tile) per pair
            for p in range(pairs):
                k_sb = io.tile([128, s], BF16, tag="k")
                v_sb = io.tile([128, s], BF16, tag="v")
                q_sb = io.tile([128, s], BF16, tag="q")
                qT_sb = io.tile([128, s], BF16, tag="qT")
                kv_sb = io.tile([128, 128], BF16, tag="kv")
                o_sb = os_pool.tile([128, s], BF16, tag="o")

                k3 = k[p].rearrange("(p n) d -> p n d", p=128)
                v3 = v[p].rearrange("(p n) d -> p n d", p=128)
                q3 = q[p].rearrange("(p n) d -> p n d", p=128)
                k_t3 = k_sb[:].rearrange("p (n d) -> p n d", d=128)
                v_t3 = v_sb[:].rearrange("p (n d) -> p n d", d=128)
                q_t3 = q_sb[:].rearrange("p (n d) -> p n d", d=128)
                nc.sync.dma_start(out=k_t3[:, ts(0, nch)], in_=k3[:, ts(0, nch)])
                nc.sync.dma_start(out=v_t3[:, ts(0, nch)], in_=v3[:, ts(0, nch)])
                # last pair's q in halves: its transpose chain starts at the
                # first half instead of waiting for the whole tile.
                qn = 2 if p == pairs - 1 else 1
                for i in range(qn):
                    qs = ts(i, nch // qn)
                    nc.sync.dma_start(out=q_t3[:, qs], in_=q3[:, qs])

                # phase A: kv[d,e] accumulated over s-chunks
                kv_ps = pskv.tile([128, 128], F32, tag="kv_ps")
                for n in range(nch):
                    nc.tensor.matmul(
                        kv_ps[:],
                        lhsT=k_sb[:, ts(n, 128)],
                        rhs=v_sb[:, ts(n, 128)],
                        start=(n == 0),
                        stop=(n == nch - 1),
                    )
                nc.vector.tensor_copy(kv_sb[:], kv_ps[:])

                def emit_T(g):
                    # transpose output dtype must match input (bf16 PSUM)
                    qt_ps = psq.tile([128, gsz * 128], BF16, tag="qt_ps")
                    for j in range(gsz):
                        n = g * gsz + j
                        nc.tensor.transpose(
                            qt_ps[:, ts(j, 128)], q_sb[:, ts(n, 128)], ident[:]
                        )
                    nc.scalar.copy(qT_sb[:, ts(g, gsz * 128)], qt_ps[:])

                def emit_B(g):
                    o_ps = pso.tile([128, gsz * 128], F32, tag="o_ps")
                    for j in range(gsz):
                        n = g * gsz + j
                        nc.tensor.matmul(
                            o_ps[:, ts(j, 128)],
                            lhsT=qT_sb[:, ts(n, 128)],
                            rhs=kv_sb[:],
                            start=True,
                            stop=True,
                        )
                    nc.vector.tensor_add(
                        o_sb[:, ts(g, gsz * 128)],
                        o_ps[:],
                        v_sb[:, ts(g, gsz * 128)],
                    )

                emit_T(0)
                for g in range(ngrp):
                    if g + 1 < ngrp:
                        emit_T(g + 1)
                    emit_B(g)

                stores.append((out[p].rearrange("(p n) d -> p n d", p=128), o_sb))

            # stores, emitted after ALL load triggers on the same (in-order)
            # Sync sequencer: their descriptors queue behind every load.
            # Last pair in halves so the final add chain overlaps the drain.
            for p, (o3, o_sb) in enumerate(stores):
                o_t3 = o_sb[:].rearrange("p (n d) -> p n d", d=128)
                if p < pairs - 1:
                    hs = bass.ds(0, nch)
                    nc.sync.dma_start(out=o3[:, hs], in_=o_t3[:, hs])
                else:
                    for h in range(2):
                        hs = bass.ds(h * (nch // 2), nch // 2)
                        nc.sync.dma_start(out=o3[:, hs], in_=o_t3[:, hs])
    nc.finalize()
    return nc


def kernel(q, k, v, _trace=False):
    bf16 = ml_dtypes.bfloat16
    q = np.ascontiguousarray(np.asarray(q, dtype=np.float32).astype(bf16)).reshape(
        B * H, S, D
    )
    k = np.ascontiguousarray(np.asarray(k, dtype=np.float32).astype(bf16)).reshape(
        B * H, S, D
    )
    v = np.ascontiguousarray(np.asarray(v, dtype=np.float32).astype(bf16)).reshape(
        B * H, S, D
    )

    nc = build_nc()
    in_maps = [
        {
            "q": q[i * PAIRS : (i + 1) * PAIRS],
            "k": k[i * PAIRS : (i + 1) * PAIRS],
            "v": v[i * PAIRS : (i + 1) * PAIRS],
        }
        for i in range(N_CORES)
    ]
    res = run_bass_kernel_spmd(nc, in_maps, core_ids=list(range(N_CORES)))
    full = np.concatenate([res.results[i]["out"] for i in range(N_CORES)], axis=0)
    out = full.reshape(B, H, S, D).astype(np.float32)
    if _trace:
        tres = [
            run_bass_kernel_spmd(
                nc,
                in_maps,
                core_ids=list(range(N_CORES)),
                trace=True,
                trace_cores=list(range(N_CORES)),
            )
            for _ in range(3)
        ]
        return out, tres
    return out
